# revision 2
# baseline (speedup 1.0000x reference)
"""GQA cross-attention kernel for Trainium2 (8 NeuronCores, Bass/Tile).

Problem: q (2,2048,16,64) f32, kv (2,2048,2,4,64) f32, key_padding_mask (2,2048)
bool.  Reference: GQA attention with additive -10000 padding bias and a causal
mask shifted by the per-batch valid key count sk, softmax over keys.

Math identical to the previous version (host-side shift makes the device
program a static causal flash-attention kernel; exp without max-subtraction;
softmax denominator via a ones-column appended to V; division on host).

Performance structure (per core, 2 head-pairs x 2 batches):
  * Row-tiled scores matmuls: head0's Q^T/K^T live in SBUF partitions 0-63,
    head1's in 64-127.  The two 64-contraction score matmuls occupy disjoint
    row-groups of the PE array (tile_position (0,0)/(64,0) auto-derived from
    base_partition) and execute CONCURRENTLY -> ~2x score throughput.
  * Chunk-outer loop (512-wide output chunks, k-tiles inner) so only one
    [65,2,512] PSUM accumulator pair is live: PSUM = 2 strip bufs (2 banks
    each) + 2 acc bufs (2 banks each) = 8 banks exactly.
  * One exp ACTIVATE per (chunk, k-tile) covering both heads' strips.
  * PV trails S by 2 units so the tensor queue never blocks on exp.
  * Big merged input DMAs split across the sync + gpsimd queues; fp16 output.
"""

import os
import math
import numpy as np

import concourse.bass as bass
import concourse.mybir as mybir
import concourse.tile as tile
from concourse import bacc
from concourse.bass_utils import run_bass_kernel_spmd

B, SQ, SK, H, HK, D = 2, 2048, 2048, 16, 4, 64
NCORES = 8
P = 128
CH = 512  # output chunk width (one PSUM bank of fp32 per head)
FP = mybir.dt.float32
FR = mybir.dt.float16
F16 = np.float16

LAST_EXEC_NS = None


def _ceil_div(a, b):
    return -(-a // b)


def _build_program(sks):
    """Build + compile the SPMD program for per-batch valid key counts sks."""
    nc = bacc.Bacc("TRN2", target_bir_lowering=False, debug=False,
                   num_devices=NCORES)

    # inputs: per pair (=batch) with head0 in partitions 0-63, head1 in 64-127
    q2_d = nc.dram_tensor("q2", [B, P, SQ], FR, kind="ExternalInput").ap()
    kTd_d = nc.dram_tensor("kTd", [B, P, SK], FR, kind="ExternalInput").ap()
    vp_d = nc.dram_tensor("vp", [B, P, (SK // P) * 65], FR,
                          kind="ExternalInput").ap()
    tri_d = nc.dram_tensor("tri", [P, P], FR, kind="ExternalInput").ap()
    out_d = nc.dram_tensor("outT", [B, 65, 2, SQ], FR,
                           kind="ExternalOutput").ap()

    EXP = mybir.ActivationFunctionType.Exp

    with tile.TileContext(nc) as tc:
        with (
            tc.tile_pool(name="const", bufs=1) as cpool,
            tc.tile_pool(name="kv", bufs=1) as kvpool,
            tc.tile_pool(name="pt", bufs=4) as ppool,
            tc.tile_pool(name="oc", bufs=3) as opool,
            tc.tile_pool(name="ps", bufs=2, space="PSUM") as spool,
            tc.tile_pool(name="pa", bufs=1, space="PSUM") as apool,
        ):
            kTd_sb = []
            q2_sb = []
            vp_sb = []
            for b in range(B):
                kTd_sb.append(kvpool.tile([P, SK], FR, name=f"kTd{b}",
                                          tag=f"kTd{b}"))
                q2_sb.append(kvpool.tile([P, SQ], FR, name=f"q2{b}",
                                         tag=f"q2{b}"))
                vp_sb.append(kvpool.tile([P, (SK // P) * 65], FR,
                                         name=f"vp{b}", tag=f"vp{b}"))
            tri_sb = cpool.tile([P, P], FR, name="tri_sb")

            # batch-0 inputs + tri from the sync queue (hardware DGE),
            # batch-1 inputs from the gpsimd queue (software DGE) so they
            # stream in during pair-0 compute without serializing startup.
            nc.sync.dma_start(kTd_sb[0][:], kTd_d[0])
            nc.sync.dma_start(q2_sb[0][:], q2_d[0])
            nc.sync.dma_start(tri_sb[:], tri_d[:])
            nc.sync.dma_start(vp_sb[0][:], vp_d[0])
            nc.gpsimd.dma_start(kTd_sb[1][:], kTd_d[1])
            nc.gpsimd.dma_start(q2_sb[1][:], q2_d[1])
            nc.gpsimd.dma_start(vp_sb[1][:], vp_d[1])

            ci = 0  # global chunk counter (acc buffer alternation)

            for p in range(B):
                U = sks[p]
                KT = _ceil_div(U, P)
                NCH = _ceil_div(U, CH)

                pending = []

                def emit_pv(ent):
                    (acc, kt, kw, off, w, pt, is_first, is_last,
                     ck, cw) = ent
                    for hh in range(2):
                        nc.tensor.matmul(
                            acc[0:65, hh, off:off + w],
                            lhsT=vp_sb[p][0:kw, 65 * kt:65 * (kt + 1)],
                            rhs=pt[0:kw, hh, off:off + w],
                            start=is_first, stop=is_last,
                            skip_group_check=True,
                        )
                    if is_last:
                        # chunk complete: evacuate PSUM and stream out
                        oc = opool.tile([65, 2, CH], FR, name="oc", tag="oc")
                        nc.vector.tensor_copy(oc[:, :, 0:cw],
                                              acc[:, :, 0:cw])
                        nc.sync.dma_start(
                            out_d[p][:, :, CH * ck:CH * ck + cw],
                            oc[:, :, 0:cw])

                for c in range(NCH):
                    u_lo = CH * c
                    u_hi = min(U, CH * (c + 1))
                    cw = u_hi - u_lo
                    ktmax = min(KT, _ceil_div(u_hi, P))
                    acc = apool.tile([65, 2, CH], FP, name="acc",
                                     tag=f"acc{ci % 2}")
                    ci += 1
                    for kt in range(ktmax):
                        kw = min(P, U - P * kt)
                        a0 = max(u_lo, P * kt)
                        w = u_hi - a0
                        off = a0 - u_lo
                        ps = spool.tile([P, 2, CH], FP, name="ps", tag="ps")
                        # two concurrent row-tiled score matmuls (one per
                        # head); each output stays inside one PSUM bank
                        for hh in range(2):
                            nc.tensor.matmul(
                                ps[0:kw, hh, off:off + w],
                                lhsT=kTd_sb[p][64 * hh:64 * hh + 64,
                                               P * kt:P * kt + kw],
                                rhs=q2_sb[p][64 * hh:64 * hh + 64, a0:a0 + w],
                                start=True, stop=True,
                                skip_group_check=True,
                            )
                        pt = ppool.tile([P, 2, CH], FR, name="pt", tag="pt")
                        nc.scalar.activation(pt[0:kw, :, off:off + w],
                                             ps[0:kw, :, off:off + w],
                                             EXP, scale=0.125)
                        if P * kt >= u_lo:
                            # diagonal block: causal triangle mask
                            dw = min(kw, w)
                            for hh in range(2):
                                nc.vector.tensor_mul(
                                    pt[0:kw, hh, off:off + dw],
                                    pt[0:kw, hh, off:off + dw],
                                    tri_sb[0:kw, 0:dw])
                        pending.append((acc, kt, kw, off, w, pt,
                                        kt == 0, kt == ktmax - 1, c, cw))
                        if len(pending) > 2:
                            emit_pv(pending.pop(0))
                for ent in pending:
                    emit_pv(ent)

    nc.compile()
    return nc


_prog_cache = {}


def _get_program(sks):
    if sks not in _prog_cache:
        _prog_cache[sks] = _build_program(sks)
    return _prog_cache[sks]


def kernel(q, kv, key_padding_mask):
    global LAST_EXEC_NS
    q = np.asarray(q, dtype=np.float32)
    kv = np.asarray(kv, dtype=np.float32)
    mask = np.asarray(key_padding_mask)

    sk = mask.sum(axis=1).astype(np.int64)  # (B,) valid key counts
    c = (SQ - sk).astype(np.int64)
    prog = _get_program((int(sk[0]), int(sk[1])))

    k_all = kv[:, :, 0]  # (B, SK, HK, D)
    v_all = kv[:, :, 1]

    tri = (np.arange(P)[None, :] >= np.arange(P)[:, None]).astype(np.float32)

    kTd_by_g = {}
    vp_by_g = {}
    for g in range(HK):
        kT = k_all[:, :, g, :].transpose(0, 2, 1)  # (B, D, SK)
        kTd = np.empty((B, P, SK), dtype=np.float32)
        kTd[:, :D, :] = kT
        kTd[:, D:, :] = kT
        kTd_by_g[g] = kTd.astype(F16)
        vpz = np.ones((B, SK, 65), dtype=np.float32)
        vpz[:, :, :64] = v_all[:, :, g, :]
        vp = vpz.reshape(B, SK // P, P, 65).transpose(0, 2, 1, 3)
        vp_by_g[g] = np.ascontiguousarray(
            vp.reshape(B, P, (SK // P) * 65)).astype(F16)

    in_maps = []
    for core in range(NCORES):
        g = core // 2
        h0 = 4 * g + 2 * (core % 2)
        q2 = np.zeros((B, P, SQ), dtype=np.float32)
        for b in range(B):
            U = int(sk[b])
            q2[b, :D, :U] = q[b, c[b]:, h0, :].T
            q2[b, D:, :U] = q[b, c[b]:, h0 + 1, :].T
        in_maps.append({
            "q2": q2.astype(F16),
            "kTd": kTd_by_g[g],
            "vp": vp_by_g[g],
            "tri": tri.astype(F16),
        })

    trace = bool(os.environ.get("BASS_KERNEL_TRACE"))
    res = run_bass_kernel_spmd(prog, in_maps, list(range(NCORES)),
                               trace=trace)
    LAST_EXEC_NS = res.exec_time_ns

    out = np.empty((B, SQ, H, D), dtype=np.float32)
    # fully-masked rows: uniform softmax over all SK keys -> mean of v
    vmean = v_all.mean(axis=1)  # (B, HK, D)
    for b in range(B):
        if c[b] > 0:
            for g in range(HK):
                for h in range(4 * g, 4 * g + 4):
                    out[b, :c[b], h, :] = vmean[b, g]

    for core in range(NCORES):
        g = core // 2
        h0 = 4 * g + 2 * (core % 2)
        o = res.results[core]["outT"].astype(np.float32)  # (B, 65, 2, SQ)
        for b in range(B):
            U = int(sk[b])
            for hh in range(2):
                num = o[b, :64, hh, :U]
                den = o[b, 64, hh, :U]
                out[b, c[b]:, h0 + hh, :] = (num / den[None, :]).T

    return out


# revision 3
# speedup vs baseline: 1.2114x; 1.2114x over previous
"""GQA cross-attention kernel for Trainium2 (8 NeuronCores, Bass/Tile).

Problem: q (2,2048,16,64) f32, kv (2,2048,2,4,64) f32, key_padding_mask (2,2048)
bool.  Reference: GQA attention with additive -10000 padding bias and a causal
mask shifted by the per-batch valid key count sk, softmax over keys.

Math identical to the previous version (host-side shift makes the device
program a static causal flash-attention kernel; exp without max-subtraction;
softmax denominator via a ones-column appended to V; division on host).

Performance structure (per core, 2 head-pairs x 2 batches):
  * Row-tiled scores matmuls: head0's Q^T/K^T live in SBUF partitions 0-63,
    head1's in 64-127.  The two 64-contraction score matmuls occupy disjoint
    row-groups of the PE array (tile_position (0,0)/(64,0) auto-derived from
    base_partition) and execute CONCURRENTLY -> ~2x score throughput.
  * Chunk-outer loop (512-wide output chunks, k-tiles inner) so only one
    [65,2,512] PSUM accumulator pair is live: PSUM = 2 strip bufs (2 banks
    each) + 2 acc bufs (2 banks each) = 8 banks exactly.
  * One exp ACTIVATE per (chunk, k-tile) covering both heads' strips.
  * PV trails S by 2 units so the tensor queue never blocks on exp.
  * Big merged input DMAs split across the sync + gpsimd queues; fp16 output.
"""

import os
import math
import numpy as np

import concourse.bass as bass
import concourse.mybir as mybir
import concourse.tile as tile
from concourse import bacc
from concourse.bass_utils import run_bass_kernel_spmd

B, SQ, SK, H, HK, D = 2, 2048, 2048, 16, 4, 64
NCORES = 8
P = 128
CH = 512  # output chunk width (one PSUM bank of fp32 per head)
FP = mybir.dt.float32
FR = mybir.dt.float16
F16 = np.float16

LAST_EXEC_NS = None


def _ceil_div(a, b):
    return -(-a // b)


def _build_program(sks):
    """Build + compile the SPMD program for per-batch valid key counts sks."""
    nc = bacc.Bacc("TRN2", target_bir_lowering=False, debug=False,
                   num_devices=NCORES)

    # inputs: per pair (=batch) with head0 in partitions 0-63, head1 in 64-127
    q2_d = nc.dram_tensor("q2", [B, P, SQ], FR, kind="ExternalInput").ap()
    kTd_d = nc.dram_tensor("kTd", [B, P, SK], FR, kind="ExternalInput").ap()
    vp_d = nc.dram_tensor("vp", [B, P, (SK // P) * 65], FR,
                          kind="ExternalInput").ap()
    tri_d = nc.dram_tensor("tri", [P, P], FR, kind="ExternalInput").ap()
    out_d = nc.dram_tensor("outT", [B, 65, 2, SQ], FR,
                           kind="ExternalOutput").ap()

    EXP = mybir.ActivationFunctionType.Exp

    with tile.TileContext(nc) as tc:
        with (
            tc.tile_pool(name="const", bufs=1) as cpool,
            tc.tile_pool(name="kv", bufs=1) as kvpool,
            tc.tile_pool(name="pt", bufs=4) as ppool,
            tc.tile_pool(name="oc", bufs=3) as opool,
            tc.tile_pool(name="ps", bufs=2, space="PSUM") as spool,
            tc.tile_pool(name="pa", bufs=1, space="PSUM") as apool,
        ):
            kTd_sb = []
            q2_sb = []
            vp_sb = []
            for b in range(B):
                kTd_sb.append(kvpool.tile([P, SK], FR, name=f"kTd{b}",
                                          tag=f"kTd{b}"))
                q2_sb.append(kvpool.tile([P, SQ], FR, name=f"q2{b}",
                                         tag=f"q2{b}"))
                vp_sb.append(kvpool.tile([P, (SK // P) * 65], FR,
                                         name=f"vp{b}", tag=f"vp{b}"))
            tri_sb = cpool.tile([P, P], FR, name="tri_sb")

            # batch-0 inputs + tri from the sync queue (hardware DGE),
            # batch-1 inputs from the gpsimd queue (software DGE) so they
            # stream in during pair-0 compute without serializing startup.
            # The first two transfers are staged (front halves first) so the
            # first matmul doesn't wait for the full 2.6MB input load: the
            # DMA engines are shared, so back-to-back whole-tensor issues
            # would all complete together.
            HF = SQ // 2
            nc.sync.dma_start(kTd_sb[0][:, 0:HF], kTd_d[0][:, 0:HF])
            nc.sync.dma_start(q2_sb[0][:, 0:HF], q2_d[0][:, 0:HF])
            nc.sync.dma_start(tri_sb[:], tri_d[:])
            nc.sync.dma_start(vp_sb[0][:, 0:520], vp_d[0][:, 0:520])
            nc.sync.dma_start(kTd_sb[0][:, HF:], kTd_d[0][:, HF:])
            nc.sync.dma_start(q2_sb[0][:, HF:], q2_d[0][:, HF:])
            nc.sync.dma_start(vp_sb[0][:, 520:], vp_d[0][:, 520:])
            nc.gpsimd.dma_start(kTd_sb[1][:], kTd_d[1])
            nc.gpsimd.dma_start(q2_sb[1][:], q2_d[1])
            nc.gpsimd.dma_start(vp_sb[1][:], vp_d[1])

            ci = 0  # global chunk counter (acc buffer alternation)

            for p in range(B):
                U = sks[p]
                KT = _ceil_div(U, P)
                NCH = _ceil_div(U, CH)

                pending = []

                def emit_pv(ent):
                    (acc, kt, kw, off, w, pt, is_first, is_last,
                     ck, cw) = ent
                    for hh in range(2):
                        nc.tensor.matmul(
                            acc[0:65, hh, off:off + w],
                            lhsT=vp_sb[p][0:kw, 65 * kt:65 * (kt + 1)],
                            rhs=pt[0:kw, hh, off:off + w],
                            start=is_first, stop=is_last,
                            skip_group_check=True,
                        )
                    if is_last:
                        # chunk complete: evacuate PSUM and stream out
                        oc = opool.tile([65, 2, CH], FR, name="oc", tag="oc")
                        nc.vector.tensor_copy(oc[:, :, 0:cw],
                                              acc[:, :, 0:cw])
                        nc.sync.dma_start(
                            out_d[p][:, :, CH * ck:CH * ck + cw],
                            oc[:, :, 0:cw])

                for c in range(NCH):
                    u_lo = CH * c
                    u_hi = min(U, CH * (c + 1))
                    cw = u_hi - u_lo
                    ktmax = min(KT, _ceil_div(u_hi, P))
                    acc = apool.tile([65, 2, CH], FP, name="acc",
                                     tag=f"acc{ci % 2}")
                    ci += 1
                    for kt in range(ktmax):
                        kw = min(P, U - P * kt)
                        a0 = max(u_lo, P * kt)
                        w = u_hi - a0
                        off = a0 - u_lo
                        ps = spool.tile([P, 2, CH], FP, name="ps", tag="ps")
                        # two concurrent row-tiled score matmuls (one per
                        # head); each output stays inside one PSUM bank
                        for hh in range(2):
                            nc.tensor.matmul(
                                ps[0:kw, hh, off:off + w],
                                lhsT=kTd_sb[p][64 * hh:64 * hh + 64,
                                               P * kt:P * kt + kw],
                                rhs=q2_sb[p][64 * hh:64 * hh + 64, a0:a0 + w],
                                start=True, stop=True,
                                skip_group_check=True,
                            )
                        pt = ppool.tile([P, 2, CH], FR, name="pt", tag="pt")
                        nc.scalar.activation(pt[0:kw, :, off:off + w],
                                             ps[0:kw, :, off:off + w],
                                             EXP, scale=0.125)
                        if P * kt >= u_lo:
                            # diagonal block: causal triangle mask
                            dw = min(kw, w)
                            for hh in range(2):
                                nc.vector.tensor_mul(
                                    pt[0:kw, hh, off:off + dw],
                                    pt[0:kw, hh, off:off + dw],
                                    tri_sb[0:kw, 0:dw])
                        pending.append((acc, kt, kw, off, w, pt,
                                        kt == 0, kt == ktmax - 1, c, cw))
                        if len(pending) > 2:
                            emit_pv(pending.pop(0))
                for ent in pending:
                    emit_pv(ent)

    nc.compile()
    return nc


_prog_cache = {}


def _get_program(sks):
    if sks not in _prog_cache:
        _prog_cache[sks] = _build_program(sks)
    return _prog_cache[sks]


def kernel(q, kv, key_padding_mask):
    global LAST_EXEC_NS
    q = np.asarray(q, dtype=np.float32)
    kv = np.asarray(kv, dtype=np.float32)
    mask = np.asarray(key_padding_mask)

    sk = mask.sum(axis=1).astype(np.int64)  # (B,) valid key counts
    c = (SQ - sk).astype(np.int64)
    prog = _get_program((int(sk[0]), int(sk[1])))

    k_all = kv[:, :, 0]  # (B, SK, HK, D)
    v_all = kv[:, :, 1]

    tri = (np.arange(P)[None, :] >= np.arange(P)[:, None]).astype(np.float32)

    kTd_by_g = {}
    vp_by_g = {}
    for g in range(HK):
        kT = k_all[:, :, g, :].transpose(0, 2, 1)  # (B, D, SK)
        kTd = np.empty((B, P, SK), dtype=np.float32)
        kTd[:, :D, :] = kT
        kTd[:, D:, :] = kT
        kTd_by_g[g] = kTd.astype(F16)
        vpz = np.ones((B, SK, 65), dtype=np.float32)
        vpz[:, :, :64] = v_all[:, :, g, :]
        vp = vpz.reshape(B, SK // P, P, 65).transpose(0, 2, 1, 3)
        vp_by_g[g] = np.ascontiguousarray(
            vp.reshape(B, P, (SK // P) * 65)).astype(F16)

    in_maps = []
    for core in range(NCORES):
        g = core // 2
        h0 = 4 * g + 2 * (core % 2)
        q2 = np.zeros((B, P, SQ), dtype=np.float32)
        for b in range(B):
            U = int(sk[b])
            q2[b, :D, :U] = q[b, c[b]:, h0, :].T
            q2[b, D:, :U] = q[b, c[b]:, h0 + 1, :].T
        in_maps.append({
            "q2": q2.astype(F16),
            "kTd": kTd_by_g[g],
            "vp": vp_by_g[g],
            "tri": tri.astype(F16),
        })

    trace = bool(os.environ.get("BASS_KERNEL_TRACE"))
    res = run_bass_kernel_spmd(prog, in_maps, list(range(NCORES)),
                               trace=trace)
    LAST_EXEC_NS = res.exec_time_ns

    out = np.empty((B, SQ, H, D), dtype=np.float32)
    # fully-masked rows: uniform softmax over all SK keys -> mean of v
    vmean = v_all.mean(axis=1)  # (B, HK, D)
    for b in range(B):
        if c[b] > 0:
            for g in range(HK):
                for h in range(4 * g, 4 * g + 4):
                    out[b, :c[b], h, :] = vmean[b, g]

    for core in range(NCORES):
        g = core // 2
        h0 = 4 * g + 2 * (core % 2)
        o = res.results[core]["outT"].astype(np.float32)  # (B, 65, 2, SQ)
        for b in range(B):
            U = int(sk[b])
            for hh in range(2):
                num = o[b, :64, hh, :U]
                den = o[b, 64, hh, :U]
                out[b, c[b]:, h0 + hh, :] = (num / den[None, :]).T

    return out


# revision 4
# speedup vs baseline: 1.2480x; 1.0303x over previous
"""GQA cross-attention kernel for Trainium2 (8 NeuronCores, Bass/Tile).

Problem: q (2,2048,16,64) f32, kv (2,2048,2,4,64) f32, key_padding_mask (2,2048)
bool.  Reference: GQA attention with additive -10000 padding bias and a causal
mask shifted by the per-batch valid key count sk, softmax over keys.

Math identical to the previous version (host-side shift makes the device
program a static causal flash-attention kernel; exp without max-subtraction;
softmax denominator via a ones-column appended to V; division on host).

Performance structure (per core, 2 head-pairs x 2 batches):
  * Row-tiled scores matmuls: head0's Q^T/K^T live in SBUF partitions 0-63,
    head1's in 64-127.  The two 64-contraction score matmuls occupy disjoint
    row-groups of the PE array (tile_position (0,0)/(64,0) auto-derived from
    base_partition) and execute CONCURRENTLY -> ~2x score throughput.
  * Chunk-outer loop (512-wide output chunks, k-tiles inner) so only one
    [65,2,512] PSUM accumulator pair is live: PSUM = 2 strip bufs (2 banks
    each) + 2 acc bufs (2 banks each) = 8 banks exactly.
  * One exp ACTIVATE per (chunk, k-tile) covering both heads' strips.
  * PV trails S by 2 units so the tensor queue never blocks on exp.
  * Big merged input DMAs split across the sync + gpsimd queues; fp16 output.
"""

import os
import math
import numpy as np

import concourse.bass as bass
import concourse.mybir as mybir
import concourse.tile as tile
from concourse import bacc
from concourse.bass_utils import run_bass_kernel_spmd

B, SQ, SK, H, HK, D = 2, 2048, 2048, 16, 4, 64
NCORES = 8
P = 128
CH = 512  # output chunk width (one PSUM bank of fp32 per head)
FP = mybir.dt.float32
FR = mybir.dt.float16
F16 = np.float16

LAST_EXEC_NS = None


def _ceil_div(a, b):
    return -(-a // b)


def _build_program(sks):
    """Build + compile the SPMD program for per-batch valid key counts sks."""
    nc = bacc.Bacc("TRN2", target_bir_lowering=False, debug=False,
                   num_devices=NCORES)

    # inputs: per pair (=batch) with head0 in partitions 0-63, head1 in 64-127
    q2_d = nc.dram_tensor("q2", [B, P, SQ], FR, kind="ExternalInput").ap()
    kTd_d = nc.dram_tensor("kTd", [B, P, SK], FR, kind="ExternalInput").ap()
    vp_d = nc.dram_tensor("vp", [B, P, (SK // P) * 65], FR,
                          kind="ExternalInput").ap()
    tri_d = nc.dram_tensor("tri", [P, P], FR, kind="ExternalInput").ap()
    out_d = nc.dram_tensor("outT", [B, 65, 2, SQ], FR,
                           kind="ExternalOutput").ap()

    EXP = mybir.ActivationFunctionType.Exp

    with tile.TileContext(nc) as tc:
        with (
            tc.tile_pool(name="const", bufs=1) as cpool,
            tc.tile_pool(name="kv", bufs=1) as kvpool,
            tc.tile_pool(name="pt", bufs=4) as ppool,
            tc.tile_pool(name="oc", bufs=3) as opool,
            tc.tile_pool(name="ps", bufs=2, space="PSUM") as spool,
            tc.tile_pool(name="pa", bufs=1, space="PSUM") as apool,
        ):
            kTd_sb = []
            q2_sb = []
            vp_sb = []
            for b in range(B):
                kTd_sb.append(kvpool.tile([P, SK], FR, name=f"kTd{b}",
                                          tag=f"kTd{b}"))
                q2_sb.append(kvpool.tile([P, SQ], FR, name=f"q2{b}",
                                         tag=f"q2{b}"))
                vp_sb.append(kvpool.tile([P, (SK // P) * 65], FR,
                                         name=f"vp{b}", tag=f"vp{b}"))
            tri_sb = cpool.tile([P, P], FR, name="tri_sb")

            # batch-0 inputs + tri from the sync queue (hardware DGE),
            # batch-1 inputs from the gpsimd queue (software DGE) so they
            # stream in during pair-0 compute without serializing startup.
            # The first two transfers are staged (front halves first) so the
            # first matmul doesn't wait for the full 2.6MB input load: the
            # DMA engines are shared, so back-to-back whole-tensor issues
            # would all complete together.
            HF = SQ // 2
            nc.sync.dma_start(kTd_sb[0][:, 0:HF], kTd_d[0][:, 0:HF])
            nc.sync.dma_start(q2_sb[0][:, 0:HF], q2_d[0][:, 0:HF])
            nc.sync.dma_start(tri_sb[:], tri_d[:])
            nc.sync.dma_start(vp_sb[0][:, 0:520], vp_d[0][:, 0:520])
            nc.sync.dma_start(kTd_sb[0][:, HF:], kTd_d[0][:, HF:])
            nc.sync.dma_start(q2_sb[0][:, HF:], q2_d[0][:, HF:])
            nc.sync.dma_start(vp_sb[0][:, 520:], vp_d[0][:, 520:])
            # batch-1 inputs also on sync, behind the batch-0 issues: they
            # transfer during early pair-0 compute (done long before ~30us
            # when pair 1 starts) without stealing DMA-engine bandwidth
            # from the startup-critical first transfers.
            nc.sync.dma_start(kTd_sb[1][:], kTd_d[1])
            nc.sync.dma_start(q2_sb[1][:], q2_d[1])
            nc.sync.dma_start(vp_sb[1][:], vp_d[1])

            ci = 0  # global chunk counter (acc buffer alternation)

            for p in range(B):
                U = sks[p]
                KT = _ceil_div(U, P)
                NCH = _ceil_div(U, CH)

                pending = []

                def emit_pv(ent):
                    (acc, kt, kw, off, w, pt, is_first, is_last,
                     ck, cw) = ent
                    for hh in range(2):
                        nc.tensor.matmul(
                            acc[0:65, hh, off:off + w],
                            lhsT=vp_sb[p][0:kw, 65 * kt:65 * (kt + 1)],
                            rhs=pt[0:kw, hh, off:off + w],
                            start=is_first, stop=is_last,
                            skip_group_check=True,
                        )
                    if is_last:
                        # chunk complete: evacuate PSUM and stream out
                        oc = opool.tile([65, 2, CH], FR, name="oc", tag="oc")
                        nc.vector.tensor_copy(oc[:, :, 0:cw],
                                              acc[:, :, 0:cw])
                        nc.sync.dma_start(
                            out_d[p][:, :, CH * ck:CH * ck + cw],
                            oc[:, :, 0:cw])

                for c in range(NCH):
                    u_lo = CH * c
                    u_hi = min(U, CH * (c + 1))
                    cw = u_hi - u_lo
                    ktmax = min(KT, _ceil_div(u_hi, P))
                    acc = apool.tile([65, 2, CH], FP, name="acc",
                                     tag=f"acc{ci % 2}")
                    ci += 1
                    for kt in range(ktmax):
                        kw = min(P, U - P * kt)
                        a0 = max(u_lo, P * kt)
                        w = u_hi - a0
                        off = a0 - u_lo
                        ps = spool.tile([P, 2, CH], FP, name="ps", tag="ps")
                        # two concurrent row-tiled score matmuls (one per
                        # head); each output stays inside one PSUM bank
                        for hh in range(2):
                            nc.tensor.matmul(
                                ps[0:kw, hh, off:off + w],
                                lhsT=kTd_sb[p][64 * hh:64 * hh + 64,
                                               P * kt:P * kt + kw],
                                rhs=q2_sb[p][64 * hh:64 * hh + 64, a0:a0 + w],
                                start=True, stop=True,
                                skip_group_check=True,
                            )
                        pt = ppool.tile([P, 2, CH], FR, name="pt", tag="pt")
                        nc.scalar.activation(pt[0:kw, :, off:off + w],
                                             ps[0:kw, :, off:off + w],
                                             EXP, scale=0.125)
                        if P * kt >= u_lo:
                            # diagonal block: causal triangle mask
                            dw = min(kw, w)
                            for hh in range(2):
                                nc.vector.tensor_mul(
                                    pt[0:kw, hh, off:off + dw],
                                    pt[0:kw, hh, off:off + dw],
                                    tri_sb[0:kw, 0:dw])
                        pending.append((acc, kt, kw, off, w, pt,
                                        kt == 0, kt == ktmax - 1, c, cw))
                        if len(pending) > 2:
                            emit_pv(pending.pop(0))
                for ent in pending:
                    emit_pv(ent)

    nc.compile()
    return nc


_prog_cache = {}


def _get_program(sks):
    if sks not in _prog_cache:
        _prog_cache[sks] = _build_program(sks)
    return _prog_cache[sks]


def kernel(q, kv, key_padding_mask):
    global LAST_EXEC_NS
    q = np.asarray(q, dtype=np.float32)
    kv = np.asarray(kv, dtype=np.float32)
    mask = np.asarray(key_padding_mask)

    sk = mask.sum(axis=1).astype(np.int64)  # (B,) valid key counts
    c = (SQ - sk).astype(np.int64)
    prog = _get_program((int(sk[0]), int(sk[1])))

    k_all = kv[:, :, 0]  # (B, SK, HK, D)
    v_all = kv[:, :, 1]

    tri = (np.arange(P)[None, :] >= np.arange(P)[:, None]).astype(np.float32)

    kTd_by_g = {}
    vp_by_g = {}
    for g in range(HK):
        kT = k_all[:, :, g, :].transpose(0, 2, 1)  # (B, D, SK)
        kTd = np.empty((B, P, SK), dtype=np.float32)
        kTd[:, :D, :] = kT
        kTd[:, D:, :] = kT
        kTd_by_g[g] = kTd.astype(F16)
        vpz = np.ones((B, SK, 65), dtype=np.float32)
        vpz[:, :, :64] = v_all[:, :, g, :]
        vp = vpz.reshape(B, SK // P, P, 65).transpose(0, 2, 1, 3)
        vp_by_g[g] = np.ascontiguousarray(
            vp.reshape(B, P, (SK // P) * 65)).astype(F16)

    in_maps = []
    for core in range(NCORES):
        g = core // 2
        h0 = 4 * g + 2 * (core % 2)
        q2 = np.zeros((B, P, SQ), dtype=np.float32)
        for b in range(B):
            U = int(sk[b])
            q2[b, :D, :U] = q[b, c[b]:, h0, :].T
            q2[b, D:, :U] = q[b, c[b]:, h0 + 1, :].T
        in_maps.append({
            "q2": q2.astype(F16),
            "kTd": kTd_by_g[g],
            "vp": vp_by_g[g],
            "tri": tri.astype(F16),
        })

    trace = bool(os.environ.get("BASS_KERNEL_TRACE"))
    res = run_bass_kernel_spmd(prog, in_maps, list(range(NCORES)),
                               trace=trace)
    LAST_EXEC_NS = res.exec_time_ns

    out = np.empty((B, SQ, H, D), dtype=np.float32)
    # fully-masked rows: uniform softmax over all SK keys -> mean of v
    vmean = v_all.mean(axis=1)  # (B, HK, D)
    for b in range(B):
        if c[b] > 0:
            for g in range(HK):
                for h in range(4 * g, 4 * g + 4):
                    out[b, :c[b], h, :] = vmean[b, g]

    for core in range(NCORES):
        g = core // 2
        h0 = 4 * g + 2 * (core % 2)
        o = res.results[core]["outT"].astype(np.float32)  # (B, 65, 2, SQ)
        for b in range(B):
            U = int(sk[b])
            for hh in range(2):
                num = o[b, :64, hh, :U]
                den = o[b, 64, hh, :U]
                out[b, c[b]:, h0 + hh, :] = (num / den[None, :]).T

    return out
